# revision 14
# baseline (speedup 1.0000x reference)
"""AVWGCN (adaptive-vertex-weight GCN) Trainium2 kernel.

Math (reference):
    adj   = relu(E @ E.T)                      # [N, N]
    Z     = x + einsum('nm,bmi->bni', adj, x)  # [B, N, I]
    W     = einsum('nd,dio->nio', E, Wp)       # per-node weights
    bias  = E @ bp                             # [N, O]
    out   = einsum('bni,nio->bno', Z, W) + bias

Sharding: nodes N are split across the 8 cores (N_loc = 256 each).  Each
core computes its node-slice of the output for ALL batches:
  - adjI_T[m, n_loc] = relu(E[m] . E_loc[n]) + I   (the +I folds the
    residual x into the graph conv: Z = (adj + I) @ x)
  - graph conv (per b): Z^T[i, n_loc] = sum_m x[b][m,i] * adjI_T[m, n_loc]
    via PE matmuls, x tile stationary, adjI_T moving (free dim 256 ->
    full-rate float32r).
  - W gen: W[n][i, o] = sum_k Wp[k,i,o] * E_loc[n,k], o-sliced PE matmuls
    (K=16), generated in 16-node chunks interleaved with the grouped GEMM.
  - grouped GEMM (per node): out[b, o] = sum_i Z^T[i, n*B+b] * W[n][i, o]
    with the Z-slice stationary, W moving; bias (E_loc @ bp, staged via a
    DRAM scratch tensor and re-read with a partition-broadcast DMA) is
    added during the PSUM eviction.

All matmuls run as float32r (fp32 data, fast fp32 mode).
"""

import sys

sys.path.insert(0, "/opt/trn_rl_repo")

import numpy as np

B, N, DIN, DOUT, ED = 64, 2048, 128, 128, 16
NCORES = 8
NLOC = N // NCORES          # 256 nodes per core
MT = N // 128               # 16 m-tiles
NCHUNK = 16                 # nodes per W-gen / ggemm chunk
NGROUP = 4                  # nodes per ggemm PSUM group
OPG = 32                    # o-slices per W-gen PSUM tile

_CACHE = {}


def _split_multi_waits(nc, mybir, maxw=1):
    """Walrus CoreV3 codegen rejects instructions carrying more than one
    sync wait ("Too many sync wait commands").  Hoist excess waits onto
    same-engine NoOps inserted just before the offending instruction —
    identical semantics, one wait per instruction."""
    ctr = 0
    for f in nc.m.functions:
        for bb in f.blocks:
            lst = bb.instructions
            out = []
            changed = False
            for inst in lst:
                si = inst.sync_info
                if si is not None and si.on_wait is not None and len(si.on_wait) > maxw:
                    waits = list(si.on_wait)
                    keep = waits[:maxw]
                    excess = waits[maxw:]
                    for j in range(0, len(excess), maxw):
                        nop = mybir.InstNoOp(name=f"waitnop_{ctr}", ins=[], outs=[])
                        ctr += 1
                        nop.engine = inst.engine
                        nop.sync_info = mybir.SyncInfo(
                            on_wait=excess[j : j + maxw], on_update=[]
                        )
                        out.append(nop)
                    inst.sync_info = mybir.SyncInfo(
                        on_wait=keep, on_update=list(si.on_update or [])
                    )
                    changed = True
                out.append(inst)
            if changed:
                bb.instructions = out
    return ctr


def _build_nc(split_waits=True):
    from contextlib import ExitStack

    import concourse.bass as bass
    from concourse import mybir
    from concourse.tile import TileContext

    f32 = mybir.dt.float32
    f32r = mybir.dt.float32r
    ALU = mybir.AluOpType

    nc = bass.Bass()
    x_d = nc.declare_dram_parameter("x_p", [B, 128, MT * DIN], f32r, isOutput=False)
    et_d = nc.declare_dram_parameter("e_t", [ED, N], f32r, isOutput=False)
    elt_d = nc.declare_dram_parameter("e_loc_t", [ED, NLOC], f32r, isOutput=False)
    wp_d = nc.declare_dram_parameter("wp", [ED, DOUT * DIN], f32r, isOutput=False)
    bp_d = nc.declare_dram_parameter("bp", [ED, DOUT], f32r, isOutput=False)
    diag_d = nc.declare_dram_parameter("diag", [N, NLOC], f32, isOutput=False)
    out_d = nc.declare_dram_parameter("out_loc", [B, NLOC, DOUT], f32, isOutput=True)

    with ExitStack() as stack:
        tc = stack.enter_context(TileContext(nc))

        # ---- tiles that live across both phases ----
        persist = stack.enter_context(tc.tile_pool(name="persist", bufs=1))
        zt = persist.tile([128, NLOC * B], f32r)          # Z^T [i, (n, b)]
        elt = persist.tile([ED, NLOC], f32r)              # E_loc^T
        nc.sync.dma_start(out=elt[:], in_=elt_d[:])
        zt_nb = zt[:].rearrange("p (n b) -> p n b", b=B)

        dram = stack.enter_context(tc.tile_pool(name="dram", bufs=1, space="DRAM"))
        bias_dram = dram.tile([NLOC, DOUT], f32)

        whalf_pool = stack.enter_context(tc.tile_pool(name="whalf", bufs=1))
        wpp = stack.enter_context(tc.tile_pool(name="wpp", bufs=2))
        psW = stack.enter_context(tc.tile_pool(name="psW", bufs=3, space="PSUM"))
        NH = NLOC // 2  # nodes per half, 128
        OST = 16        # o-slices per streamed wp tile

        def wgen(nh):
            """W for nodes [nh*NH, (nh+1)*NH): wh[i, (j, o)].  Wp is
            streamed from DRAM in OST-o-slice tiles; every matmul operand
            sits at base partition 0."""
            wh = whalf_pool.tile([128, NH * DOUT], f32r, tag="whalf")
            wh_jo = wh[:].rearrange("p (j o) -> p j o", o=DOUT)
            for og in range(DOUT // OST):
                wpt = wpp.tile([ED, OST * DIN], f32r, tag="wpt")
                nc.sync.dma_start(
                    out=wpt[:], in_=wp_d[:, og * OST * DIN : (og + 1) * OST * DIN]
                )
                for o4 in range(OST // 4):
                    pw = psW.tile([128, 4 * NH], f32)
                    for e in range(4):
                        ol = o4 * 4 + e
                        nc.tensor.matmul(
                            pw[:, e * NH : (e + 1) * NH],
                            lhsT=wpt[:, ol * DIN : (ol + 1) * DIN],
                            rhs=elt[:, nh * NH : (nh + 1) * NH],
                            start=True,
                            stop=True,
                        )
                    o0 = og * OST + o4 * 4
                    nc.scalar.copy(
                        out=wh_jo[:, :, o0 : o0 + 4].rearrange("p j e -> p e j"),
                        in_=pw[:].rearrange("p (e j) -> p e j", e=4),
                    )
            return wh

        # ================= phase A: adjacency + bias + graph conv ========
        with ExitStack() as pa:
            constsA = pa.enter_context(tc.tile_pool(name="constsA", bufs=1))
            et = constsA.tile([ED, N], f32r)
            bp = constsA.tile([ED, DOUT], f32r)
            adj = constsA.tile([128, MT * NLOC], f32r)    # adjI_T m-tiles
            bias_sb = constsA.tile([128, (NLOC // 128) * DOUT], f32)
            nc.sync.dma_start(out=et[:], in_=et_d[:])
            nc.sync.dma_start(out=bp[:], in_=bp_d[:])

            diag_pool = pa.enter_context(tc.tile_pool(name="diag", bufs=2))
            psA = pa.enter_context(tc.tile_pool(name="psA", bufs=1, space="PSUM"))

            # bias = E_loc @ bp -> DRAM scratch (re-read broadcast in phase B)
            pbias = psA.tile([128, (NLOC // 128) * DOUT], f32)
            for h in range(NLOC // 128):
                nc.tensor.matmul(
                    pbias[:, h * DOUT : (h + 1) * DOUT],
                    lhsT=elt[:, h * 128 : (h + 1) * 128],
                    rhs=bp[:],
                    start=True,
                    stop=True,
                )
            nc.scalar.copy(out=bias_sb[:], in_=pbias[:])
            nc.sync.dma_start(
                out=bias_dram[:].rearrange("(h p) o -> p h o", p=128),
                in_=bias_sb[:].rearrange("p (h o) -> p h o", o=DOUT),
            )

            # adjI_T[m-tile] = relu(E_mtile^T . E_loc) + diag
            for mt in range(MT):
                d_t = diag_pool.tile([128, NLOC], f32)
                nc.sync.dma_start(out=d_t[:], in_=diag_d[mt * 128 : (mt + 1) * 128, :])
                pa_t = psA.tile([128, NLOC], f32)
                nc.tensor.matmul(
                    pa_t[:],
                    lhsT=et[:, mt * 128 : (mt + 1) * 128],
                    rhs=elt[:],
                    start=True,
                    stop=True,
                )
                nc.vector.scalar_tensor_tensor(
                    out=adj[:, mt * NLOC : (mt + 1) * NLOC],
                    in0=pa_t[:],
                    scalar=0.0,
                    in1=d_t[:],
                    op0=ALU.max,
                    op1=ALU.add,
                )

            # W half-0 generation hides under the HBM-bound graph conv
            wh0 = wgen(0)

            # graph conv: per b, Z^T[i, n_loc] = sum_mt x[b][mt] . adjI_T[mt]
            xp = pa.enter_context(tc.tile_pool(name="xp", bufs=2))
            psZ = pa.enter_context(tc.tile_pool(name="psZ", bufs=3, space="PSUM"))
            for b in range(B):
                xt = xp.tile([128, MT * DIN], f32r)
                nc.sync.dma_start(out=xt[:], in_=x_d[b])
                pz = psZ.tile([128, NLOC], f32)
                for mt in range(MT):
                    nc.tensor.matmul(
                        pz[:],
                        lhsT=xt[:, mt * DIN : (mt + 1) * DIN],
                        rhs=adj[:, mt * NLOC : (mt + 1) * NLOC],
                        start=(mt == 0),
                        stop=(mt == MT - 1),
                    )
                nc.vector.tensor_copy(out=zt_nb[:, :, b], in_=pz[:])

        # ================= phase B: grouped GEMM (+ W half-1 gen) ========
        with ExitStack() as pb_:
            ostage = pb_.enter_context(tc.tile_pool(name="ostage", bufs=2))
            biasp = pb_.enter_context(tc.tile_pool(name="biasp", bufs=2))
            psO = pb_.enter_context(tc.tile_pool(name="psO", bufs=4, space="PSUM"))

            def ggemm(nh, wh):
                for ch in range(NH // NCHUNK):
                    n0 = nh * NH + ch * NCHUNK
                    bt = biasp.tile([B, NCHUNK, DOUT], f32, tag="bt")
                    nc.sync.dma_start(
                        out=bt[:],
                        in_=bias_dram[n0 : n0 + NCHUNK, :].partition_broadcast(B),
                    )
                    ot = ostage.tile([B, NCHUNK, DOUT], f32, tag="ot")
                    for g in range(NCHUNK // NGROUP):
                        po = psO.tile([B, NGROUP * DOUT], f32)
                        for q in range(NGROUP):
                            j_loc = ch * NCHUNK + g * NGROUP + q
                            n_loc = nh * NH + j_loc
                            nc.tensor.matmul(
                                po[:, q * DOUT : (q + 1) * DOUT],
                                lhsT=zt[:, n_loc * B : (n_loc + 1) * B],
                                rhs=wh[:, j_loc * DOUT : (j_loc + 1) * DOUT],
                                start=True,
                                stop=True,
                            )
                        nc.vector.tensor_add(
                            ot[:, g * NGROUP : (g + 1) * NGROUP, :],
                            po[:].rearrange("b (j o) -> b j o", o=DOUT),
                            bt[:, g * NGROUP : (g + 1) * NGROUP, :],
                        )
                    nc.sync.dma_start(out=out_d[:, n0 : n0 + NCHUNK, :], in_=ot[:])

            ggemm(0, wh0)
            wh1 = wgen(1)
            ggemm(1, wh1)

    if split_waits:
        _split_multi_waits(nc, mybir, maxw=1)
    return nc


def _get_nc():
    if "nc" not in _CACHE:
        _CACHE["nc"] = _build_nc()
    return _CACHE["nc"]


def _prep_inputs(x, node_embeddings, weights_pool, bias_pool):
    x = np.asarray(x, dtype=np.float32)
    # x_p[b, p, mt*DIN + i] = x[b, mt*128 + p, i] — one contiguous 32KB
    # run per SBUF partition
    x_p = np.ascontiguousarray(
        x.reshape(B, MT, 128, DIN).transpose(0, 2, 1, 3).reshape(B, 128, MT * DIN)
    )
    E = np.asarray(node_embeddings, dtype=np.float32)
    Wp = np.asarray(weights_pool, dtype=np.float32)
    bp = np.ascontiguousarray(np.asarray(bias_pool, dtype=np.float32))
    e_t = np.ascontiguousarray(E.T)
    # wp[k, o*DIN + i] = Wp[k, i, o]
    wp_host = np.ascontiguousarray(Wp.transpose(0, 2, 1).reshape(ED, DOUT * DIN))
    in_maps = []
    for c in range(NCORES):
        lo = c * NLOC
        e_loc_t = np.ascontiguousarray(E[lo : lo + NLOC].T)
        diag = np.zeros((N, NLOC), dtype=np.float32)
        diag[np.arange(lo, lo + NLOC), np.arange(NLOC)] = 1.0
        in_maps.append(
            {
                "x_p": x_p,
                "e_t": e_t,
                "e_loc_t": e_loc_t,
                "wp": wp_host,
                "bp": bp,
                "diag": diag,
            }
        )
    return in_maps


def kernel(x, node_embeddings, weights_pool, bias_pool, _return_results=False, **run_kwargs):
    from concourse.bass_utils import run_bass_kernel_spmd

    in_maps = _prep_inputs(x, node_embeddings, weights_pool, bias_pool)
    nc = _get_nc()
    res = run_bass_kernel_spmd(nc, in_maps, core_ids=list(range(NCORES)), **run_kwargs)
    out = np.concatenate(
        [res.results[c]["out_loc"] for c in range(NCORES)], axis=1
    )
    if _return_results:
        return out, res
    return out


# revision 15
# speedup vs baseline: 1.0767x; 1.0767x over previous
"""AVWGCN (adaptive-vertex-weight GCN) Trainium2 kernel.

Math (reference):
    adj   = relu(E @ E.T)                      # [N, N]
    Z     = x + einsum('nm,bmi->bni', adj, x)  # [B, N, I]
    W     = einsum('nd,dio->nio', E, Wp)       # per-node weights
    bias  = E @ bp                             # [N, O]
    out   = einsum('bni,nio->bno', Z, W) + bias

Sharding: nodes N are split across the 8 cores (N_loc = 256 each).  Each
core computes its node-slice of the output for ALL batches:
  - adjI_T[m, n_loc] = relu(E[m] . E_loc[n]) + I   (the +I folds the
    residual x into the graph conv: Z = (adj + I) @ x)
  - graph conv (per b): Z^T[i, n_loc] = sum_m x[b][m,i] * adjI_T[m, n_loc]
    via PE matmuls, x tile stationary, adjI_T moving (free dim 256 ->
    full-rate float32r).
  - W gen: W[n][i, o] = sum_k Wp[k,i,o] * E_loc[n,k], o-sliced PE matmuls
    (K=16), generated in 16-node chunks interleaved with the grouped GEMM.
  - grouped GEMM (per node): out[b, o] = sum_i Z^T[i, n*B+b] * W[n][i, o]
    with the Z-slice stationary, W moving; bias (E_loc @ bp, staged via a
    DRAM scratch tensor and re-read with a partition-broadcast DMA) is
    added during the PSUM eviction.

All matmuls run as float32r (fp32 data, fast fp32 mode).
"""

import sys

sys.path.insert(0, "/opt/trn_rl_repo")

import numpy as np

B, N, DIN, DOUT, ED = 64, 2048, 128, 128, 16
NCORES = 8
NLOC = N // NCORES          # 256 nodes per core
MT = N // 128               # 16 m-tiles
NCHUNK = 16                 # nodes per W-gen / ggemm chunk
NGROUP = 4                  # nodes per ggemm PSUM group
OPG = 32                    # o-slices per W-gen PSUM tile

_CACHE = {}


def _split_multi_waits(nc, mybir, maxw=1):
    """Walrus CoreV3 codegen rejects instructions carrying more than one
    sync wait ("Too many sync wait commands").  Hoist excess waits onto
    same-engine NoOps inserted just before the offending instruction —
    identical semantics, one wait per instruction."""
    ctr = 0
    for f in nc.m.functions:
        for bb in f.blocks:
            lst = bb.instructions
            out = []
            changed = False
            for inst in lst:
                si = inst.sync_info
                if si is not None and si.on_wait is not None and len(si.on_wait) > maxw:
                    waits = list(si.on_wait)
                    keep = waits[:maxw]
                    excess = waits[maxw:]
                    for j in range(0, len(excess), maxw):
                        nop = mybir.InstNoOp(name=f"waitnop_{ctr}", ins=[], outs=[])
                        ctr += 1
                        nop.engine = inst.engine
                        nop.sync_info = mybir.SyncInfo(
                            on_wait=excess[j : j + maxw], on_update=[]
                        )
                        out.append(nop)
                    inst.sync_info = mybir.SyncInfo(
                        on_wait=keep, on_update=list(si.on_update or [])
                    )
                    changed = True
                out.append(inst)
            if changed:
                bb.instructions = out
    return ctr


def _build_nc(split_waits=True):
    from contextlib import ExitStack

    import concourse.bass as bass
    from concourse import mybir
    from concourse.tile import TileContext

    f32 = mybir.dt.float32
    f32r = mybir.dt.float32r
    ALU = mybir.AluOpType

    nc = bass.Bass()
    x_d = nc.declare_dram_parameter("x_p", [B, 128, MT * DIN], f32r, isOutput=False)
    et_d = nc.declare_dram_parameter("e_t", [ED, N], f32r, isOutput=False)
    elt_d = nc.declare_dram_parameter("e_loc_t", [ED, NLOC], f32r, isOutput=False)
    wp_d = nc.declare_dram_parameter("wp", [ED, DOUT * DIN], f32r, isOutput=False)
    bp_d = nc.declare_dram_parameter("bp", [ED, DOUT], f32r, isOutput=False)
    diag_d = nc.declare_dram_parameter("diag", [N, NLOC], f32, isOutput=False)
    out_d = nc.declare_dram_parameter("out_loc", [B, NLOC, DOUT], f32, isOutput=True)

    with ExitStack() as stack:
        tc = stack.enter_context(TileContext(nc))

        # ---- tiles that live across both phases ----
        persist = stack.enter_context(tc.tile_pool(name="persist", bufs=1))
        zt = persist.tile([128, NLOC * B], f32r)          # Z^T [i, (n, b)]
        elt = persist.tile([ED, NLOC], f32r)              # E_loc^T
        nc.sync.dma_start(out=elt[:], in_=elt_d[:])
        zt_nb = zt[:].rearrange("p (n b) -> p n b", b=B)

        dram = stack.enter_context(tc.tile_pool(name="dram", bufs=1, space="DRAM"))
        bias_dram = dram.tile([NLOC, DOUT], f32)

        whalf_pool = stack.enter_context(tc.tile_pool(name="whalf", bufs=1))
        wpp = stack.enter_context(tc.tile_pool(name="wpp", bufs=1))
        psW = stack.enter_context(tc.tile_pool(name="psW", bufs=3, space="PSUM"))
        NH = NLOC // 2  # nodes per half, 128
        OST = 16        # o-slices per streamed wp tile

        def wgen(nh):
            """W for nodes [nh*NH, (nh+1)*NH): wh[i, (j, o)].  Wp is
            streamed from DRAM in OST-o-slice tiles; every matmul operand
            sits at base partition 0."""
            wh = whalf_pool.tile([128, NH * DOUT], f32r, tag="whalf")
            wh_jo = wh[:].rearrange("p (j o) -> p j o", o=DOUT)
            for og in range(DOUT // OST):
                wpt = wpp.tile([ED, OST * DIN], f32r, tag="wpt")
                nc.sync.dma_start(
                    out=wpt[:], in_=wp_d[:, og * OST * DIN : (og + 1) * OST * DIN]
                )
                for o4 in range(OST // 4):
                    pw = psW.tile([128, 4 * NH], f32)
                    for e in range(4):
                        ol = o4 * 4 + e
                        nc.tensor.matmul(
                            pw[:, e * NH : (e + 1) * NH],
                            lhsT=wpt[:, ol * DIN : (ol + 1) * DIN],
                            rhs=elt[:, nh * NH : (nh + 1) * NH],
                            start=True,
                            stop=True,
                        )
                    o0 = og * OST + o4 * 4
                    nc.scalar.copy(
                        out=wh_jo[:, :, o0 : o0 + 4].rearrange("p j e -> p e j"),
                        in_=pw[:].rearrange("p (e j) -> p e j", e=4),
                    )
            return wh

        # ================= phase A: adjacency + bias + graph conv ========
        with ExitStack() as pa:
            constsA = pa.enter_context(tc.tile_pool(name="constsA", bufs=1))
            et = constsA.tile([ED, N], f32r)
            bp = constsA.tile([ED, DOUT], f32r)
            adj = constsA.tile([128, MT * NLOC], f32r)    # adjI_T m-tiles
            bias_sb = constsA.tile([128, (NLOC // 128) * DOUT], f32)
            nc.sync.dma_start(out=et[:], in_=et_d[:])
            nc.sync.dma_start(out=bp[:], in_=bp_d[:])

            diag_pool = pa.enter_context(tc.tile_pool(name="diag", bufs=1))
            psA = pa.enter_context(tc.tile_pool(name="psA", bufs=1, space="PSUM"))

            # bias = E_loc @ bp -> DRAM scratch (re-read broadcast in phase B)
            pbias = psA.tile([128, (NLOC // 128) * DOUT], f32)
            for h in range(NLOC // 128):
                nc.tensor.matmul(
                    pbias[:, h * DOUT : (h + 1) * DOUT],
                    lhsT=elt[:, h * 128 : (h + 1) * 128],
                    rhs=bp[:],
                    start=True,
                    stop=True,
                )
            nc.scalar.copy(out=bias_sb[:], in_=pbias[:])
            nc.sync.dma_start(
                out=bias_dram[:].rearrange("(h p) o -> p h o", p=128),
                in_=bias_sb[:].rearrange("p (h o) -> p h o", o=DOUT),
            )

            # adjI_T[m-tile] = relu(E_mtile^T . E_loc) + diag
            for mt in range(MT):
                d_t = diag_pool.tile([128, NLOC], f32)
                nc.sync.dma_start(out=d_t[:], in_=diag_d[mt * 128 : (mt + 1) * 128, :])
                pa_t = psA.tile([128, NLOC], f32)
                nc.tensor.matmul(
                    pa_t[:],
                    lhsT=et[:, mt * 128 : (mt + 1) * 128],
                    rhs=elt[:],
                    start=True,
                    stop=True,
                )
                nc.vector.scalar_tensor_tensor(
                    out=adj[:, mt * NLOC : (mt + 1) * NLOC],
                    in0=pa_t[:],
                    scalar=0.0,
                    in1=d_t[:],
                    op0=ALU.max,
                    op1=ALU.add,
                )

            # W half-0 generation hides under the HBM-bound graph conv
            wh0 = wgen(0)

            # graph conv: per b, Z^T[i, n_loc] = sum_mt x[b][mt] . adjI_T[mt]
            xp = pa.enter_context(tc.tile_pool(name="xp", bufs=3))
            psZ = pa.enter_context(tc.tile_pool(name="psZ", bufs=3, space="PSUM"))
            HM = MT * DIN // 2
            for b in range(B):
                xt = xp.tile([128, MT * DIN], f32r)
                nc.sync.dma_start(out=xt[:, :HM], in_=x_d[b, :, :HM])
                nc.sync.dma_start(out=xt[:, HM:], in_=x_d[b, :, HM:])
                pz = psZ.tile([128, NLOC], f32)
                for mt in range(MT):
                    nc.tensor.matmul(
                        pz[:],
                        lhsT=xt[:, mt * DIN : (mt + 1) * DIN],
                        rhs=adj[:, mt * NLOC : (mt + 1) * NLOC],
                        start=(mt == 0),
                        stop=(mt == MT - 1),
                    )
                nc.vector.tensor_copy(out=zt_nb[:, :, b], in_=pz[:])

        # ================= phase B: grouped GEMM (+ W half-1 gen) ========
        with ExitStack() as pb_:
            ostage = pb_.enter_context(tc.tile_pool(name="ostage", bufs=2))
            biasp = pb_.enter_context(tc.tile_pool(name="biasp", bufs=2))
            psO = pb_.enter_context(tc.tile_pool(name="psO", bufs=4, space="PSUM"))

            def ggemm(nh, wh):
                for ch in range(NH // NCHUNK):
                    n0 = nh * NH + ch * NCHUNK
                    bt = biasp.tile([B, NCHUNK, DOUT], f32, tag="bt")
                    nc.sync.dma_start(
                        out=bt[:],
                        in_=bias_dram[n0 : n0 + NCHUNK, :].partition_broadcast(B),
                    )
                    ot = ostage.tile([B, NCHUNK, DOUT], f32, tag="ot")
                    for g in range(NCHUNK // NGROUP):
                        po = psO.tile([B, NGROUP * DOUT], f32)
                        for q in range(NGROUP):
                            j_loc = ch * NCHUNK + g * NGROUP + q
                            n_loc = nh * NH + j_loc
                            nc.tensor.matmul(
                                po[:, q * DOUT : (q + 1) * DOUT],
                                lhsT=zt[:, n_loc * B : (n_loc + 1) * B],
                                rhs=wh[:, j_loc * DOUT : (j_loc + 1) * DOUT],
                                start=True,
                                stop=True,
                            )
                        nc.vector.tensor_add(
                            ot[:, g * NGROUP : (g + 1) * NGROUP, :],
                            po[:].rearrange("b (j o) -> b j o", o=DOUT),
                            bt[:, g * NGROUP : (g + 1) * NGROUP, :],
                        )
                    nc.sync.dma_start(out=out_d[:, n0 : n0 + NCHUNK, :], in_=ot[:])

            ggemm(0, wh0)
            wh1 = wgen(1)
            ggemm(1, wh1)

    if split_waits:
        _split_multi_waits(nc, mybir, maxw=1)
    return nc


def _get_nc():
    if "nc" not in _CACHE:
        _CACHE["nc"] = _build_nc()
    return _CACHE["nc"]


def _prep_inputs(x, node_embeddings, weights_pool, bias_pool):
    x = np.asarray(x, dtype=np.float32)
    # x_p[b, p, mt*DIN + i] = x[b, mt*128 + p, i] — one contiguous 32KB
    # run per SBUF partition
    x_p = np.ascontiguousarray(
        x.reshape(B, MT, 128, DIN).transpose(0, 2, 1, 3).reshape(B, 128, MT * DIN)
    )
    E = np.asarray(node_embeddings, dtype=np.float32)
    Wp = np.asarray(weights_pool, dtype=np.float32)
    bp = np.ascontiguousarray(np.asarray(bias_pool, dtype=np.float32))
    e_t = np.ascontiguousarray(E.T)
    # wp[k, o*DIN + i] = Wp[k, i, o]
    wp_host = np.ascontiguousarray(Wp.transpose(0, 2, 1).reshape(ED, DOUT * DIN))
    in_maps = []
    for c in range(NCORES):
        lo = c * NLOC
        e_loc_t = np.ascontiguousarray(E[lo : lo + NLOC].T)
        diag = np.zeros((N, NLOC), dtype=np.float32)
        diag[np.arange(lo, lo + NLOC), np.arange(NLOC)] = 1.0
        in_maps.append(
            {
                "x_p": x_p,
                "e_t": e_t,
                "e_loc_t": e_loc_t,
                "wp": wp_host,
                "bp": bp,
                "diag": diag,
            }
        )
    return in_maps


def kernel(x, node_embeddings, weights_pool, bias_pool, _return_results=False, **run_kwargs):
    from concourse.bass_utils import run_bass_kernel_spmd

    in_maps = _prep_inputs(x, node_embeddings, weights_pool, bias_pool)
    nc = _get_nc()
    res = run_bass_kernel_spmd(nc, in_maps, core_ids=list(range(NCORES)), **run_kwargs)
    out = np.concatenate(
        [res.results[c]["out_loc"] for c in range(NCORES)], axis=1
    )
    if _return_results:
        return out, res
    return out


# revision 16
# speedup vs baseline: 1.0985x; 1.0203x over previous
"""AVWGCN (adaptive-vertex-weight GCN) Trainium2 kernel.

Math (reference):
    adj   = relu(E @ E.T)                      # [N, N]
    Z     = x + einsum('nm,bmi->bni', adj, x)  # [B, N, I]
    W     = einsum('nd,dio->nio', E, Wp)       # per-node weights
    bias  = E @ bp                             # [N, O]
    out   = einsum('bni,nio->bno', Z, W) + bias

Sharding: nodes N are split across the 8 cores (N_loc = 256 each).  Each
core computes its node-slice of the output for ALL batches:
  - adjI_T[m, n_loc] = relu(E[m] . E_loc[n]) + I   (the +I folds the
    residual x into the graph conv: Z = (adj + I) @ x)
  - graph conv (per b): Z^T[i, n_loc] = sum_m x[b][m,i] * adjI_T[m, n_loc]
    via PE matmuls, x tile stationary, adjI_T moving (free dim 256 ->
    full-rate float32r).
  - W gen: W[n][i, o] = sum_k Wp[k,i,o] * E_loc[n,k], o-sliced PE matmuls
    (K=16), generated in 16-node chunks interleaved with the grouped GEMM.
  - grouped GEMM (per node): out[b, o] = sum_i Z^T[i, n*B+b] * W[n][i, o]
    with the Z-slice stationary, W moving; bias (E_loc @ bp, staged via a
    DRAM scratch tensor and re-read with a partition-broadcast DMA) is
    added during the PSUM eviction.

All matmuls run as float32r (fp32 data, fast fp32 mode).
"""

import sys

sys.path.insert(0, "/opt/trn_rl_repo")

import numpy as np

B, N, DIN, DOUT, ED = 64, 2048, 128, 128, 16
NCORES = 8
NLOC = N // NCORES          # 256 nodes per core
MT = N // 128               # 16 m-tiles
NCHUNK = 16                 # nodes per W-gen / ggemm chunk
NGROUP = 4                  # nodes per ggemm PSUM group
OPG = 32                    # o-slices per W-gen PSUM tile

_CACHE = {}


def _split_multi_waits(nc, mybir, maxw=1):
    """Walrus CoreV3 codegen rejects instructions carrying more than one
    sync wait ("Too many sync wait commands").  Hoist excess waits onto
    same-engine NoOps inserted just before the offending instruction —
    identical semantics, one wait per instruction."""
    ctr = 0
    for f in nc.m.functions:
        for bb in f.blocks:
            lst = bb.instructions
            out = []
            changed = False
            for inst in lst:
                si = inst.sync_info
                if si is not None and si.on_wait is not None and len(si.on_wait) > maxw:
                    waits = list(si.on_wait)
                    keep = waits[:maxw]
                    excess = waits[maxw:]
                    for j in range(0, len(excess), maxw):
                        nop = mybir.InstNoOp(name=f"waitnop_{ctr}", ins=[], outs=[])
                        ctr += 1
                        nop.engine = inst.engine
                        nop.sync_info = mybir.SyncInfo(
                            on_wait=excess[j : j + maxw], on_update=[]
                        )
                        out.append(nop)
                    inst.sync_info = mybir.SyncInfo(
                        on_wait=keep, on_update=list(si.on_update or [])
                    )
                    changed = True
                out.append(inst)
            if changed:
                bb.instructions = out
    return ctr


def _build_nc(split_waits=True):
    from contextlib import ExitStack

    import concourse.bass as bass
    from concourse import mybir
    from concourse.tile import TileContext

    f32 = mybir.dt.float32
    f32r = mybir.dt.float32r
    ALU = mybir.AluOpType

    nc = bass.Bass()
    x_d = nc.declare_dram_parameter("x_p", [B, 128, MT * DIN], f32r, isOutput=False)
    et_d = nc.declare_dram_parameter("e_t", [ED, N], f32r, isOutput=False)
    elt_d = nc.declare_dram_parameter("e_loc_t", [ED, NLOC], f32r, isOutput=False)
    wp_d = nc.declare_dram_parameter("wp", [ED, DOUT * DIN], f32r, isOutput=False)
    bp_d = nc.declare_dram_parameter("bp", [ED, DOUT], f32r, isOutput=False)
    diag_d = nc.declare_dram_parameter("diag", [N, NLOC], f32, isOutput=False)
    out_d = nc.declare_dram_parameter("out_loc", [B, NLOC, DOUT], f32, isOutput=True)

    with ExitStack() as stack:
        tc = stack.enter_context(TileContext(nc))

        # ---- tiles that live across both phases ----
        persist = stack.enter_context(tc.tile_pool(name="persist", bufs=1))
        zt = persist.tile([128, NLOC * B], f32r)          # Z^T [i, (n, b)]
        elt = persist.tile([ED, NLOC], f32r)              # E_loc^T
        nc.sync.dma_start(out=elt[:], in_=elt_d[:])
        zt_nb = zt[:].rearrange("p (n b) -> p n b", b=B)

        dram = stack.enter_context(tc.tile_pool(name="dram", bufs=1, space="DRAM"))
        bias_dram = dram.tile([NLOC, DOUT], f32)

        whalf_pool = stack.enter_context(tc.tile_pool(name="whalf", bufs=1))
        wpp = stack.enter_context(tc.tile_pool(name="wpp", bufs=1))
        psW = stack.enter_context(tc.tile_pool(name="psW", bufs=3, space="PSUM"))
        NH = NLOC // 2  # nodes per half, 128
        OST = 16        # o-slices per streamed wp tile

        def wgen(nh):
            """W for nodes [nh*NH, (nh+1)*NH): wh[i, (j, o)].  Wp is
            streamed from DRAM in OST-o-slice tiles; every matmul operand
            sits at base partition 0."""
            wh = whalf_pool.tile([128, NH * DOUT], f32r, tag="whalf")
            wh_jo = wh[:].rearrange("p (j o) -> p j o", o=DOUT)
            for og in range(DOUT // OST):
                wpt = wpp.tile([ED, OST * DIN], f32r, tag="wpt")
                nc.sync.dma_start(
                    out=wpt[:], in_=wp_d[:, og * OST * DIN : (og + 1) * OST * DIN]
                )
                for o4 in range(OST // 4):
                    pw = psW.tile([128, 4 * NH], f32)
                    for e in range(4):
                        ol = o4 * 4 + e
                        nc.tensor.matmul(
                            pw[:, e * NH : (e + 1) * NH],
                            lhsT=wpt[:, ol * DIN : (ol + 1) * DIN],
                            rhs=elt[:, nh * NH : (nh + 1) * NH],
                            start=True,
                            stop=True,
                        )
                    o0 = og * OST + o4 * 4
                    nc.scalar.copy(
                        out=wh_jo[:, :, o0 : o0 + 4].rearrange("p j e -> p e j"),
                        in_=pw[:].rearrange("p (e j) -> p e j", e=4),
                    )
            return wh

        # ================= phase A: adjacency + bias + graph conv ========
        with ExitStack() as pa:
            constsA = pa.enter_context(tc.tile_pool(name="constsA", bufs=1))
            et = constsA.tile([ED, N], f32r)
            bp = constsA.tile([ED, DOUT], f32r)
            adj = constsA.tile([128, MT * NLOC], f32r)    # adjI_T m-tiles
            bias_sb = constsA.tile([128, (NLOC // 128) * DOUT], f32)
            nc.sync.dma_start(out=et[:], in_=et_d[:])
            nc.sync.dma_start(out=bp[:], in_=bp_d[:])

            diag_pool = pa.enter_context(tc.tile_pool(name="diag", bufs=1))
            psA = pa.enter_context(tc.tile_pool(name="psA", bufs=1, space="PSUM"))

            # bias = E_loc @ bp -> DRAM scratch (re-read broadcast in phase B)
            pbias = psA.tile([128, (NLOC // 128) * DOUT], f32, tag="pa_t")
            for h in range(NLOC // 128):
                nc.tensor.matmul(
                    pbias[:, h * DOUT : (h + 1) * DOUT],
                    lhsT=elt[:, h * 128 : (h + 1) * 128],
                    rhs=bp[:],
                    start=True,
                    stop=True,
                )
            nc.scalar.copy(out=bias_sb[:], in_=pbias[:])
            nc.sync.dma_start(
                out=bias_dram[:].rearrange("(h p) o -> p h o", p=128),
                in_=bias_sb[:].rearrange("p (h o) -> p h o", o=DOUT),
            )

            # adjI_T[m-tile] = relu(E_mtile^T . E_loc) + diag
            for mt in range(MT):
                d_t = diag_pool.tile([128, NLOC], f32)
                nc.sync.dma_start(out=d_t[:], in_=diag_d[mt * 128 : (mt + 1) * 128, :])
                pa_t = psA.tile([128, NLOC], f32)
                nc.tensor.matmul(
                    pa_t[:],
                    lhsT=et[:, mt * 128 : (mt + 1) * 128],
                    rhs=elt[:],
                    start=True,
                    stop=True,
                )
                nc.vector.scalar_tensor_tensor(
                    out=adj[:, mt * NLOC : (mt + 1) * NLOC],
                    in0=pa_t[:],
                    scalar=0.0,
                    in1=d_t[:],
                    op0=ALU.max,
                    op1=ALU.add,
                )

            # W half-0 generation hides under the HBM-bound graph conv
            wh0 = wgen(0)

            # graph conv: per b, Z^T[i, n_loc] = sum_mt x[b][mt] . adjI_T[mt]
            xp = pa.enter_context(tc.tile_pool(name="xp", bufs=3))
            psZ = pa.enter_context(tc.tile_pool(name="psZ", bufs=4, space="PSUM"))
            QM = MT * DIN // 4
            for b in range(B):
                xt = xp.tile([128, MT * DIN], f32r)
                for qq in range(4):
                    nc.sync.dma_start(
                        out=xt[:, qq * QM : (qq + 1) * QM],
                        in_=x_d[b, :, qq * QM : (qq + 1) * QM],
                    )
                pz = psZ.tile([128, NLOC], f32)
                for mt in range(MT):
                    nc.tensor.matmul(
                        pz[:],
                        lhsT=xt[:, mt * DIN : (mt + 1) * DIN],
                        rhs=adj[:, mt * NLOC : (mt + 1) * NLOC],
                        start=(mt == 0),
                        stop=(mt == MT - 1),
                    )
                nc.vector.tensor_copy(out=zt_nb[:, :, b], in_=pz[:])

        # ================= phase B: grouped GEMM (+ W half-1 gen) ========
        with ExitStack() as pb_:
            ostage = pb_.enter_context(tc.tile_pool(name="ostage", bufs=2))
            biasp = pb_.enter_context(tc.tile_pool(name="biasp", bufs=2))
            psO = pb_.enter_context(tc.tile_pool(name="psO", bufs=4, space="PSUM"))

            def ggemm(nh, wh):
                for ch in range(NH // NCHUNK):
                    n0 = nh * NH + ch * NCHUNK
                    bt = biasp.tile([B, NCHUNK, DOUT], f32, tag="bt")
                    nc.sync.dma_start(
                        out=bt[:],
                        in_=bias_dram[n0 : n0 + NCHUNK, :].partition_broadcast(B),
                    )
                    ot = ostage.tile([B, NCHUNK, DOUT], f32, tag="ot")
                    for g in range(NCHUNK // NGROUP):
                        po = psO.tile([B, NGROUP * DOUT], f32)
                        for q in range(NGROUP):
                            j_loc = ch * NCHUNK + g * NGROUP + q
                            n_loc = nh * NH + j_loc
                            nc.tensor.matmul(
                                po[:, q * DOUT : (q + 1) * DOUT],
                                lhsT=zt[:, n_loc * B : (n_loc + 1) * B],
                                rhs=wh[:, j_loc * DOUT : (j_loc + 1) * DOUT],
                                start=True,
                                stop=True,
                            )
                        nc.vector.tensor_add(
                            ot[:, g * NGROUP : (g + 1) * NGROUP, :],
                            po[:].rearrange("b (j o) -> b j o", o=DOUT),
                            bt[:, g * NGROUP : (g + 1) * NGROUP, :],
                        )
                    nc.sync.dma_start(out=out_d[:, n0 : n0 + NCHUNK, :], in_=ot[:])

            ggemm(0, wh0)
            wh1 = wgen(1)
            ggemm(1, wh1)

    if split_waits:
        _split_multi_waits(nc, mybir, maxw=1)
    return nc


def _get_nc():
    if "nc" not in _CACHE:
        _CACHE["nc"] = _build_nc()
    return _CACHE["nc"]


def _prep_inputs(x, node_embeddings, weights_pool, bias_pool):
    x = np.asarray(x, dtype=np.float32)
    # x_p[b, p, mt*DIN + i] = x[b, mt*128 + p, i] — one contiguous 32KB
    # run per SBUF partition
    x_p = np.ascontiguousarray(
        x.reshape(B, MT, 128, DIN).transpose(0, 2, 1, 3).reshape(B, 128, MT * DIN)
    )
    E = np.asarray(node_embeddings, dtype=np.float32)
    Wp = np.asarray(weights_pool, dtype=np.float32)
    bp = np.ascontiguousarray(np.asarray(bias_pool, dtype=np.float32))
    e_t = np.ascontiguousarray(E.T)
    # wp[k, o*DIN + i] = Wp[k, i, o]
    wp_host = np.ascontiguousarray(Wp.transpose(0, 2, 1).reshape(ED, DOUT * DIN))
    in_maps = []
    for c in range(NCORES):
        lo = c * NLOC
        e_loc_t = np.ascontiguousarray(E[lo : lo + NLOC].T)
        diag = np.zeros((N, NLOC), dtype=np.float32)
        diag[np.arange(lo, lo + NLOC), np.arange(NLOC)] = 1.0
        in_maps.append(
            {
                "x_p": x_p,
                "e_t": e_t,
                "e_loc_t": e_loc_t,
                "wp": wp_host,
                "bp": bp,
                "diag": diag,
            }
        )
    return in_maps


def kernel(x, node_embeddings, weights_pool, bias_pool, _return_results=False, **run_kwargs):
    from concourse.bass_utils import run_bass_kernel_spmd

    in_maps = _prep_inputs(x, node_embeddings, weights_pool, bias_pool)
    nc = _get_nc()
    res = run_bass_kernel_spmd(nc, in_maps, core_ids=list(range(NCORES)), **run_kwargs)
    out = np.concatenate(
        [res.results[c]["out_loc"] for c in range(NCORES)], axis=1
    )
    if _return_results:
        return out, res
    return out
